# revision 1
# baseline (speedup 1.0000x reference)
"""Causal shaped attention kernel for Trainium2 (8 NeuronCores).

y = beta * softmax(causal(q k^T / 8)) @ v + alpha * Id @ v - gamma * MC @ v
  with q,k = x @ w_attn.T split, v = x, Id = softmax(eye(T)), MC = causal row-mean.

Sharding: (batch, head-group) across 8 cores: core c -> b = c//2, heads
h0 = (c%2)*8 .. h0+8.  Each core computes y[b, :, h0*64 : h0*64+512].

Id@v and MC@v have closed forms (no T x T materialization):
  Id@v[i] = ((e-1) v[i] + colsum(v)) / (e+T-1)
  MC@v[i] = cumsum(v)[i] / (i+1)

On-device layout (per core):
  xT   [128, 8, 2048]   x[b]^T by 128-wide c-chunks (PE-transposed)
  WTq  [128, 4, 8, 128] per head-pair p, c-chunk ci: [Wq_even^T | Wq_odd^T]
  WTk  same for k
  qkT  [128, 4, 2, 2048] pair p: partitions 0:64 even head, 64:128 odd head;
                         [.., 0, :] = q^T, [.., 1, :] = k^T
  vones [128, 8, 16, 65] per head hh, j-tile J: [v | 1]  (AV lhsT)
  static [128, 16, 512]  k1*v + k2*colsum - gamma*cumsum/(i+1), combine addend

Attention per (head, i-strip g of 512): S^T tiles [j=128, i<=512] via PE
(K=64, fp32r), exp on ACT (scale=1/8 folded in), causal diag masked by tril
multiply, AV matmul lhsT=[v|1] gives y^T and rowsum in one pass, PE transpose
back, normalize + add static, DMA out.
"""

import sys

if "/opt/trn_rl_repo" not in sys.path:
    sys.path.insert(0, "/opt/trn_rl_repo")

import math

import numpy as np

import concourse.bass as bass
import concourse.mybir as mybir
import concourse.tile as tile
from concourse import bacc
from concourse.bass_utils import run_bass_kernel_spmd

F32 = mybir.dt.float32
F32R = mybir.dt.float32r
AF = mybir.ActivationFunctionType
OP = mybir.AluOpType

N_CORES = 8
B, T, C = 4, 2048, 1024
H, HD = 16, 64
NHC = 8          # heads per core
NT = T // 128    # 16 j/i tiles
NS = 4           # i-strips of 512
CONSTS_W = 276   # 128 tril + 16 negipg + k1 + k2 + kb + pad + 128 ident

_NC_CACHE = {}


def r(ap):
    return ap.bitcast(F32R)


def emit(nc, tc, xb, wqk, consts, yout):
    ctx_pools = []

    def pool(name, **kw):
        p = tc.alloc_tile_pool(name=name, **kw)
        ctx_pools.append(p)
        return p

    cpool = pool("cpool", bufs=1)
    ps = pool("ps", bufs=2, space="PSUM")

    cons = cpool.tile([128, CONSTS_W], F32, name="cons")
    nc.sync.dma_start(out=cons[:], in_=consts[:])
    tril = cons[:, 0:128]
    ident = cons[:, 148:276]
    negipg = cons[:, 128:144]      # [128, 16] : -gamma/(i+1)
    k1c = cons[:, 144:145]
    k2c = cons[:, 145:146]
    kbc = cons[:, 146:147]
    trilr = cpool.tile([128, 128], F32R, name="trilr")
    nc.vector.tensor_copy(out=trilr[:], in_=tril)
    ones_row = trilr[0:1, 0:128]   # tril row 0 == all ones (K=1 lhsT)
    ones_col = trilr[:, 127:128]   # tril col 127 == all ones [128, 1]

    qkp = pool("qkp", bufs=1)
    qkT = qkp.tile([128, 4, 2, 2048], F32R, name="qkT")

    # ---------------- phase A: transposes of W and x ----------------
    wtp = pool("wtp", bufs=1)
    WTq = wtp.tile([128, 4, 8, 128], F32R, name="WTq")
    WTk = wtp.tile([128, 4, 8, 128], F32R, name="WTk")
    xT = wtp.tile([128, 8, 2048], F32R, name="xT")

    ldp = pool("ldp", bufs=2)
    for p in range(4):
        for qk, WT in ((0, WTq), (1, WTk)):
            tw = ldp.tile([128, 1024], F32, name="tw", tag="tw")
            nc.sync.dma_start(out=tw[:], in_=wqk[qk * 512 + p * 128: qk * 512 + (p + 1) * 128, :])
            for cg in range(2):  # groups of 4 c-chunks
                pst = ps.tile([128, 512], F32, name="pst", tag="ps")
                for k in range(4):
                    ci = cg * 4 + k
                    nc.tensor.transpose(pst[:, k * 128:(k + 1) * 128],
                                        tw[:, ci * 128:(ci + 1) * 128], ident)
                nc.scalar.copy(out=WT[:, p, cg * 4:(cg + 1) * 4, :], in_=pst[:])
    for tt in range(NT):
        tx = ldp.tile([128, 1024], F32, name="tx", tag="tx")
        nc.sync.dma_start(out=tx[:], in_=xb[tt * 128:(tt + 1) * 128, :])
        for cg in range(2):
            pst = ps.tile([128, 512], F32, name="pstx", tag="ps")
            for k in range(4):
                ci = cg * 4 + k
                nc.tensor.transpose(pst[:, k * 128:(k + 1) * 128],
                                    tx[:, ci * 128:(ci + 1) * 128], ident)
            nc.scalar.copy(out=xT[:, cg * 4:(cg + 1) * 4, tt * 128:(tt + 1) * 128],
                           in_=pst[:].rearrange("p (a b) -> p a b", a=4))

    # ---------------- phase B: projections -> qkT ----------------
    for p in range(4):
        for qk, WT in ((0, WTq), (1, WTk)):
            for s in range(NS):
                pj = ps.tile([128, 512], F32, name="pj", tag="ps")
                for ci in range(8):
                    nc.tensor.matmul(pj[:], r(WT[:, p, ci, :]),
                                     r(xT[:, ci, s * 512:(s + 1) * 512]),
                                     start=(ci == 0), stop=(ci == 7))
                nc.vector.tensor_copy(out=qkT[:, p, qk, s * 512:(s + 1) * 512], in_=pj[:])

    # ---------------- phase B2: vones, colsum/cumsum, static ----------------
    ldp.release()
    ctx_pools.remove(ldp)
    wtp.release()
    ctx_pools.remove(wtp)
    b2 = pool("b2", bufs=1)
    b2s = pool("b2s", bufs=1)
    vones = b2.tile([128, NHC, NT, 65], F32R, name="vones")
    # strided gather of v columns: vones[p, hh, J, d] = xb[J*128+p, hh*64+d]
    nc.vector.memset(vones[:].bitcast(F32), 1.0)
    for hh in range(NHC):
        xs_view = xb[:, hh * 64:(hh + 1) * 64].rearrange("(J p) d -> p J d", p=128)
        nc.sync.dma_start(out=vones[:, hh, :, 0:64], in_=xs_view.bitcast(F32R))

    colb = b2.tile([128, 512], F32, name="colb")
    run = b2.tile([1, 512], F32R, name="run")       # exclusive prefix of tile colsums
    runs = b2.tile([1, 512], F32, name="runs")      # k2-scaled total (staging)
    static = b2.tile([128, NT, 512], F32, name="static")

    # pass 1: total colsum -> colb
    nc.vector.memset(run[:].bitcast(F32), 0.0)
    for I in range(NT):
        cp = ps.tile([1, 512], F32, name="cp", tag="cs", bufs=1)
        for hh in range(NHC):
            nc.tensor.matmul(cp[0:1, hh * 64:(hh + 1) * 64], r(ones_col),
                             r(vones[:, hh, I, 0:64]), start=True, stop=True)
        nc.vector.tensor_add(run[0:1, :], run[0:1, :], cp[0:1, :])
    nc.vector.tensor_scalar(out=runs[:], in0=run[0:1, :].bitcast(F32),
                            scalar1=cons[0:1, 145:146], scalar2=None, op0=OP.mult)
    nc.gpsimd.partition_broadcast(colb[:], runs[0:1, :])

    # pass 2: running exclusive prefix + cumsum + static
    nc.vector.memset(run[:].bitcast(F32), 0.0)
    for I in range(NT):
        cu = ps.tile([128, 512], F32, name="cu", tag="ps")
        nc.tensor.matmul(cu[:], r(ones_row), r(run[0:1, :]), start=True, stop=False)
        for hh in range(NHC):
            nc.tensor.matmul(cu[:, hh * 64:(hh + 1) * 64], r(trilr[:]),
                             r(vones[:, hh, I, 0:64]), start=False,
                             stop=(hh == NHC - 1))
        cp = ps.tile([1, 512], F32, name="cp2", tag="cs", bufs=1)
        for hh in range(NHC):
            nc.tensor.matmul(cp[0:1, hh * 64:(hh + 1) * 64], r(ones_col),
                             r(vones[:, hh, I, 0:64]), start=True, stop=True)
        nc.vector.tensor_add(run[0:1, :], run[0:1, :], cp[0:1, :])
        nc.vector.scalar_tensor_tensor(
            out=static[:, I, :].rearrange("p (h d) -> p h d", h=NHC),
            in0=vones[:, :, I, 0:64],
            scalar=k1c, in1=colb[:].rearrange("p (h d) -> p h d", h=NHC),
            op0=OP.mult, op1=OP.add)
        nc.vector.scalar_tensor_tensor(
            out=static[:, I, :], in0=cu[:], scalar=negipg[:, I:I + 1],
            in1=static[:, I, :], op0=OP.mult, op1=OP.add)

    # ---------------- phase C: attention per (head, i-strip) ----------------
    cp3 = pool("cp3", bufs=1)
    ptA = cp3.tile([128, 8, 512], F32R, name="ptA")
    ptB = cp3.tile([128, 8, 512], F32R, name="ptB")
    ysp = pool("ysp", bufs=2)

    for p in range(4):
        for half in range(2):
            hh = 2 * p + half
            base = half * 64
            qT = qkT[base:base + 64, p, 0, :]
            kT = qkT[base:base + 64, p, 1, :]
            for g in range(NS):
                nj = 4 * g + 4
                yps = ps.tile([128, 512], F32, name="yps", tag="yps", bufs=2)
                pts = []
                sidx = hh * NS + g

                def ptof(J):
                    if nj <= 8:
                        return (ptA if sidx % 2 == 0 else ptB)[:, J, :]
                    return ptA[:, J, :] if J < 8 else ptB[:, J - 8, :]

                def pt2of(J):
                    if nj <= 8:
                        return (ptA if sidx % 2 == 0 else ptB)[:, J:J + 2, :]
                    return ptA[:, J:J + 2, :] if J < 8 else ptB[:, J - 8:J - 6, :]

                J = 0
                while J < nj:
                    if J + 1 <= 4 * g and J % 2 == 0:
                        # two full-width j-tiles: one 2-bank psum, one exp
                        st2 = ps.tile([128, 2, 512], F32, name="st2", tag="ps2", bufs=1)
                        for u in range(2):
                            nc.tensor.matmul(
                                st2[:, u, :], r(kT[:, (J + u) * 128:(J + u + 1) * 128]),
                                r(qT[:, g * 512:(g + 1) * 512]),
                                start=True, stop=True)
                        pt2 = pt2of(J)
                        nc.scalar.activation(out=pt2, in_=st2[:],
                                             func=AF.Exp, scale=0.125)
                        for u in range(2):
                            if J + u == 4 * g:
                                nc.gpsimd.tensor_mul(pt2[:, u, 0:128],
                                                     pt2[:, u, 0:128], tril)
                            pts.append((pt2[:, u, :], 0))
                        J += 2
                        continue
                    i_off = max(0, 128 * J - 512 * g)
                    st = ps.tile([128, 512], F32, name="st", tag="ps")
                    nc.tensor.matmul(
                        st[:, i_off:512], r(kT[:, J * 128:(J + 1) * 128]),
                        r(qT[:, g * 512 + i_off:(g + 1) * 512]),
                        start=True, stop=True)
                    pt = ptof(J)
                    nc.scalar.activation(out=pt[:, i_off:512], in_=st[:, i_off:512],
                                         func=AF.Exp, scale=0.125)
                    if i_off > 0 or J == 4 * g:
                        # diagonal tile: keep j <= i only
                        nc.gpsimd.tensor_mul(pt[:, i_off:i_off + 128],
                                             pt[:, i_off:i_off + 128], tril)
                    pts.append((pt, i_off))
                    J += 1
                for J in range(nj):
                    pt, i_off = pts[J]
                    nc.tensor.matmul(
                        yps[0:65, i_off:512], r(vones[:, hh, J, :]),
                        r(pt[:, i_off:512]),
                        start=(J == 0), stop=(J == nj - 1), skip_group_check=True)
                # evacuate y^T [65, 512], transpose back to [i, 65]
                ysb = ysp.tile([65, 512], F32, name="ysb", tag="ysb")
                nc.vector.tensor_copy(out=ysb[:], in_=yps[0:65, :])
                tp = ps.tile([128, 260], F32, name="tp", tag="tp", bufs=1)
                for k in range(4):
                    nc.tensor.transpose(tp[:, k * 65:(k + 1) * 65],
                                        ysb[:, k * 128:(k + 1) * 128], ident[0:65, 0:65])
                rc4 = ysp.tile([128, 4], F32, name="rc4", tag="rc4")
                nc.vector.reciprocal(out=rc4[:], in_=tp[:, 64:260:65])
                nc.vector.tensor_scalar(out=rc4[:], in0=rc4[:], scalar1=kbc,
                                        scalar2=None, op0=OP.mult)
                yo = ysp.tile([128, 4, 64], F32, name="yo", tag="yo")
                for k in range(4):
                    nc.vector.scalar_tensor_tensor(
                        out=yo[:, k, :], in0=tp[:, k * 65:k * 65 + 64],
                        scalar=rc4[:, k:k + 1],
                        in1=static[:, 4 * g + k, hh * 64:(hh + 1) * 64],
                        op0=OP.mult, op1=OP.add)
                nc.sync.dma_start(
                    out=yout[g * 512:(g + 1) * 512, hh * 64:(hh + 1) * 64]
                    .rearrange("(k p) d -> p k d", p=128),
                    in_=yo[:])

    for p in reversed(ctx_pools):
        p.release()


def build_nc():
    if "nc" in _NC_CACHE:
        return _NC_CACHE["nc"]
    nc = bacc.Bacc("TRN2", target_bir_lowering=False)
    xb = nc.declare_dram_parameter("xb", [T, C], F32, isOutput=False)
    wqk = nc.declare_dram_parameter("wqk", [C, C], F32, isOutput=False)
    consts = nc.declare_dram_parameter("consts", [128, CONSTS_W], F32, isOutput=False)
    yout = nc.declare_dram_parameter("yout", [T, 512], F32, isOutput=True)
    with tile.TileContext(nc) as tc:
        emit(nc, tc, xb, wqk, consts, yout)
    nc.compile()
    _NC_CACHE["nc"] = nc
    return nc


def make_consts(alpha, beta, gamma):
    D = math.e + T - 1
    k1 = alpha * (math.e - 1.0) / D
    k2 = alpha / D
    cons = np.zeros((128, CONSTS_W), dtype=np.float32)
    jj = np.arange(128)
    cons[:, 0:128] = (jj[:, None] <= jj[None, :]).astype(np.float32)  # tril mask
    for I in range(16):
        cons[:, 128 + I] = -gamma / (128.0 * I + jj + 1.0)
    cons[:, 144] = k1
    cons[:, 145] = k2
    cons[:, 146] = beta
    cons[:, 148:276] = np.eye(128, dtype=np.float32)
    return cons


def kernel(x, w_attn, alpha, beta, gamma, _trace=False):
    x = np.asarray(x, dtype=np.float32)
    w_attn = np.asarray(w_attn, dtype=np.float32)
    alpha = float(np.asarray(alpha))
    beta = float(np.asarray(beta))
    gamma = float(np.asarray(gamma))

    nc = build_nc()
    cons = make_consts(alpha, beta, gamma)
    in_maps = []
    for c in range(N_CORES):
        b, h0 = c // 2, (c % 2) * 8
        wqk = np.concatenate(
            [w_attn[h0 * 64: h0 * 64 + 512], w_attn[C + h0 * 64: C + h0 * 64 + 512]], axis=0)
        # rotate columns of x and w so this core's v-block sits at columns 0:512
        # (the projection q,k = x @ w.T is invariant to a consistent column roll)
        c0 = h0 * 64
        xb_r = np.roll(x[b], -c0, axis=1)
        wqk_r = np.roll(wqk, -c0, axis=1)
        in_maps.append({"xb": np.ascontiguousarray(xb_r),
                        "wqk": np.ascontiguousarray(wqk_r), "consts": cons})
    res = run_bass_kernel_spmd(nc, in_maps, list(range(N_CORES)), trace=_trace)
    y = np.empty((B, T, C), dtype=np.float32)
    for c in range(N_CORES):
        b, h0 = c // 2, (c % 2) * 8
        y[b, :, h0 * 64: h0 * 64 + 512] = res.results[c]["yout"]
    if _trace:
        kernel.last_exec_time_ns = res.exec_time_ns
    return y



# revision 10
# speedup vs baseline: 1.8062x; 1.8062x over previous
"""Causal shaped attention kernel for Trainium2 (8 NeuronCores).

y = beta * softmax(causal(q k^T / 8)) @ v + alpha * Id @ v - gamma * MC @ v
  with q,k = x @ w_attn.T split, v = x, Id = softmax(eye(T)), MC = causal row-mean.

Sharding: (batch, head-group) across 8 cores: core c -> b = c//2, heads
h0 = (c%2)*8 .. h0+8.  Each core computes y[b, :, h0*64 : h0*64+512].

Id@v and MC@v have closed forms (no T x T materialization):
  Id@v[i] = ((e-1) v[i] + colsum(v)) / (e+T-1)
  MC@v[i] = cumsum(v)[i] / (i+1)

v2 design (vs v1 baseline at ~583us):
 - x^T, W^T and the vones ([v|1]) AV-lhsT are packed on HOST in bf16:
   no on-device transposes, half the input DMA bytes, half the LDWEIGHTS.
 - B2 (Id/MC "static" term) has no serial prefix recurrence: per-tile
   column sums go into a [16,512] PSUM tile (one matmul per row), then a
   per-tile exclusive prefix is formed by 16 independent selector matmuls
   (selcol[J, I*128+p] = J<I), + one tril matmul for the local cumsum.
 - Phase C is software-pipelined at tile-pair granularity: the QK+exp
   stream of unit u+1 interleaves with the AV stream of unit u, and
   projection chains for head pair p+1 are injected into the PE stream
   while heads of pair p run attention, keeping PE dense (p-state!).
 - Everything 2-byte on the PE: qkT, pt (exp output), vones in bf16;
   PSUM stays fp32.
"""

import sys

if "/opt/trn_rl_repo" not in sys.path:
    sys.path.insert(0, "/opt/trn_rl_repo")

import math

import numpy as np
import ml_dtypes

import concourse.bass as bass
import concourse.mybir as mybir
import concourse.tile as tile
from concourse import bacc
from concourse.bass_utils import run_bass_kernel_spmd

F32 = mybir.dt.float32
BF16 = mybir.dt.bfloat16
AF = mybir.ActivationFunctionType
OP = mybir.AluOpType
BFNP = ml_dtypes.bfloat16

N_CORES = 8
B, T, C = 4, 2048, 1024
H, HD = 16, 64
NHC = 8          # heads per core
NT = T // 128    # 16 j/i tiles
NS = 4           # i-strips of 512

# bf16 consts layout: tril 128 | ztril 256 | ident 128 | ones_col 1 | selcol 17*128
CB_TRIL = 0
CB_ZTRIL = 128
CB_IDENT = 384
CB_ONEC = 512
CB_SEL = 513
CB_W = 513 + 17 * 128          # 2689
# f32 consts: negipg 16 | k1 | k2 | kb | pad | ident128
CF_IDENT = 20
CF_W = 20 + 128

_NC_CACHE = {}


def emit(nc, tc, xT, WT, vones, consb, consf, yout):
    pools = []

    def pool(name, **kw):
        p = tc.alloc_tile_pool(name=name, **kw)
        pools.append(p)
        return p

    # ---- persistent SBUF ----
    cpool = pool("cpool", bufs=1)
    cb = cpool.tile([128, CB_W], BF16, name="cb")
    cf = cpool.tile([128, CF_W], F32, name="cf")
    xTs = cpool.tile([128, 8, 2048], BF16, name="xTs")
    WTs = cpool.tile([128, 8, 1024], BF16, name="WTs")
    vos = cpool.tile([128, NHC, NT, 65], BF16, name="vos")
    qkT = cpool.tile([128, 4, 2, 2048], BF16, name="qkT")
    static = cpool.tile([128, NT, 512], F32, name="static")
    colb = cpool.tile([128, 512], F32, name="colb")
    run = cpool.tile([1, 512], BF16, name="run")

    nc.sync.dma_start(out=cb[:], in_=consb[:])
    nc.sync.dma_start(out=cf[:], in_=consf[:])
    nc.sync.dma_start(out=vos[:], in_=vones[:].rearrange(
        "p (h J d) -> p h J d", h=NHC, J=NT))
    nc.sync.dma_start(out=WTs[:], in_=WT[:].rearrange("p (c d) -> p c d", c=8))
    nc.sync.dma_start(out=xTs[:], in_=xT[:].rearrange("p (c i) -> p c i", c=8))

    tril = cb[:, CB_TRIL:CB_TRIL + 128]
    ztril = cb[:, CB_ZTRIL:CB_ZTRIL + 256]
    identf = cf[:, CF_IDENT:CF_IDENT + 128]
    ones_col = cb[:, CB_ONEC:CB_ONEC + 1]
    k1c = cf[:, 16:17]
    k2c = cf[:, 17:18]
    kbc = cf[:, 18:19]

    def selcol(i):
        return cb[0:16, CB_SEL + i * 128: CB_SEL + (i + 1) * 128]

    # ---- PSUM pools: proj pj (2 banks) + B2 (5, released before phase C) ----
    pspj = pool("pspj", bufs=2, space="PSUM")
    psB2 = pool("psB2", bufs=1, space="PSUM")

    # ================= B2: static = k1*v + k2*colsum - g/(i+1)*cumsum ======
    ones_row = tril[0:1, 0:128]   # tril row 0 == all ones (K=1 lhsT)
    nc.vector.memset(run[:], 0.0)
    # per-tile column sums (one matmul each), exclusive tile-prefix kept as a
    # rolling [1,512] bf16 row (DVE in-place adds; /(i+1) makes rounding moot)
    for I in range(NT):
        cp = psB2.tile([1, 512], F32, name="cp", tag="cp", bufs=2)
        nc.tensor.matmul(cp[0:1, :], ones_col, vos[:, :, I, 0:64],
                         start=True, stop=True)
        cu = psB2.tile([128, 512], F32, name="cu", tag="cu", bufs=2)
        nc.tensor.matmul(cu[:], ones_row, run[0:1, :], start=True, stop=False)
        nc.tensor.matmul(cu[:], tril, vos[:, :, I, 0:64], start=False, stop=True)
        nc.vector.tensor_add(run[0:1, :], run[0:1, :], cp[0:1, :])
        # static_I = negipg_I * cu  (cu bank recycled 2 tiles later)
        nc.vector.tensor_scalar(out=static[:, I, :], in0=cu[:],
                                scalar1=cf[:, I:I + 1], scalar2=None,
                                op0=OP.mult)
    colb_ps = psB2.tile([128, 512], F32, name="colb_ps", tag="colb")
    nc.tensor.matmul(colb_ps[:], ones_row, run[0:1, :], start=True, stop=True)
    nc.vector.tensor_scalar(out=colb[:], in0=colb_ps[:], scalar1=k2c,
                            scalar2=None, op0=OP.mult)
    for I in range(NT):
        # static_I += k1*v_I
        nc.vector.scalar_tensor_tensor(
            out=static[:, I, :].rearrange("p (h d) -> p h d", h=NHC),
            in0=vos[:, :, I, 0:64], scalar=k1c,
            in1=static[:, I, :].rearrange("p (h d) -> p h d", h=NHC),
            op0=OP.mult, op1=OP.add)
    for I in range(NT):
        # static_I += colb
        nc.vector.tensor_add(static[:, I, :], static[:, I, :], colb[:])
    psB2.release()
    pools.remove(psB2)

    # ================= projection groups =================
    # group (p4, qk, s): 8-matmul K-chain into pj, DVE-evacuate to qkT (bf16)
    def proj_group(p4, qk, s):
        def go():
            pj = pspj.tile([128, 512], F32, name="pj", tag="pj")
            for ci in range(8):
                nc.tensor.matmul(
                    pj[:], WTs[:, ci, qk * 512 + p4 * 128: qk * 512 + (p4 + 1) * 128],
                    xTs[:, ci, s * 512:(s + 1) * 512],
                    start=(ci == 0), stop=(ci == 7))
            nc.vector.tensor_copy(out=qkT[:, p4, qk, s * 512:(s + 1) * 512],
                                  in_=pj[:])
        return go

    proj_sched = [[proj_group(p4, qk, s) for qk in range(2) for s in range(NS)]
                  for p4 in range(4)]

    # emit pair 0 projection up-front
    for go in proj_sched[0]:
        go()
    proj_done = 1  # pairs fully emitted
    proj_pend = []  # pending closures for pair proj_done (being drip-fed)

    # ================= phase C =================
    psst = pool("psst", bufs=2, space="PSUM")   # st2 [128,2,512] -> 4 banks
    psyp = pool("psyp", bufs=2, space="PSUM")   # yps [128,512]   -> 2 banks
    ptp = pool("ptp", bufs=10)                  # pt2 ring (bf16)
    ysbp = pool("ysbp", bufs=2)
    rcp = pool("rcp", bufs=2)
    yop = pool("yop", bufs=2)

    units = [(h, g) for h in range(NHC) for g in range(NS)]

    def qk_groups(u):
        """List of closures; each emits 2 QK matmuls + 1 exp (+ masks) and
        returns the pt2 tile for the AV stage (stored into pt_map)."""
        h, g = u
        p4, base = h // 2, (h % 2) * 64
        qT = qkT[base:base + 64, p4, 0, :]
        kT = qkT[base:base + 64, p4, 1, :]
        i0 = g * 512
        out = []

        def full_pair(fp):
            def go():
                st2 = psst.tile([128, 2, 512], F32, name="st2", tag="st2")
                for u2 in range(2):
                    J = 2 * fp + u2
                    nc.tensor.matmul(st2[:, u2, :], kT[:, J * 128:(J + 1) * 128],
                                     qT[:, i0:i0 + 512], start=True, stop=True,
                                     skip_group_check=True)
                pt2 = ptp.tile([128, 2, 512], BF16, name="pt2", tag="pt2")
                nc.scalar.activation(out=pt2[:], in_=st2[:], func=AF.Exp,
                                     scale=0.125)
                pt_map[(u, fp)] = pt2
            return go

        def diagA():
            def go():
                st2 = psst.tile([128, 2, 512], F32, name="st2", tag="st2")
                J = 4 * g
                nc.tensor.matmul(st2[:, 0, :], kT[:, J * 128:(J + 1) * 128],
                                 qT[:, i0:i0 + 512], start=True, stop=True,
                                 skip_group_check=True)
                nc.tensor.matmul(st2[:, 1, 128:512],
                                 kT[:, (J + 1) * 128:(J + 2) * 128],
                                 qT[:, i0 + 128:i0 + 512], start=True, stop=True,
                                 skip_group_check=True)
                pt2 = ptp.tile([128, 2, 512], BF16, name="pt2", tag="pt2")
                # whole [2,512] exp'd in one inst; [1, 0:128] is stale junk
                # (finite) that AV never reads.
                nc.scalar.activation(out=pt2[:], in_=st2[:], func=AF.Exp,
                                     scale=0.125)
                nc.gpsimd.tensor_mul(pt2[:, 0, 0:128], pt2[:, 0, 0:128], tril)
                nc.gpsimd.tensor_mul(pt2[:, 1, 128:256], pt2[:, 1, 128:256], tril)
                pt_map[(u, 2 * g)] = pt2
            return go

        def diagB():
            def go():
                st2 = psst.tile([128, 2, 512], F32, name="st2", tag="st2")
                for u2 in range(2):
                    J = 4 * g + 2 + u2
                    nc.tensor.matmul(st2[:, u2, 256:512],
                                     kT[:, J * 128:(J + 1) * 128],
                                     qT[:, i0 + 256:i0 + 512], start=True,
                                     stop=True, skip_group_check=True)
                pt2 = ptp.tile([128, 2, 512], BF16, name="pt2", tag="pt2")
                nc.scalar.activation(out=pt2[:, :, 256:512],
                                     in_=st2[:, :, 256:512], func=AF.Exp,
                                     scale=0.125)
                nc.gpsimd.tensor_mul(pt2[:, 0, 256:384], pt2[:, 0, 256:384], tril)
                nc.gpsimd.tensor_mul(pt2[:, 1, 256:512], pt2[:, 1, 256:512], ztril)
                pt_map[(u, 2 * g + 1)] = pt2
            return go

        for fp in range(2 * g):
            out.append(full_pair(fp))
        out.append(diagA())
        out.append(diagB())
        return out

    pt_map = {}
    yps_map = {}

    def av_groups(u):
        h, g = u
        ngr = 2 * g + 2

        def grp(q):
            def go():
                if q == 0:
                    yps_map[u] = psyp.tile([128, 512], F32, name="yps", tag="yps")
                yps = yps_map[u]
                pt2 = pt_map.pop((u, q))
                if q < 2 * g:          # full pair
                    for u2 in range(2):
                        J = 2 * q + u2
                        nc.tensor.matmul(yps[0:65, :], vos[:, h, J, :],
                                         pt2[:, u2, :],
                                         start=(q == 0 and u2 == 0),
                                         stop=(q == ngr - 1 and u2 == 1),
                                         skip_group_check=True)
                elif q == 2 * g:       # diagA
                    nc.tensor.matmul(yps[0:65, :], vos[:, h, 4 * g, :],
                                     pt2[:, 0, :], start=(q == 0), stop=False,
                                     skip_group_check=True)
                    nc.tensor.matmul(yps[0:65, 128:512], vos[:, h, 4 * g + 1, :],
                                     pt2[:, 1, 128:512], start=False, stop=False,
                                     skip_group_check=True)
                else:                  # diagB
                    nc.tensor.matmul(yps[0:65, 256:512], vos[:, h, 4 * g + 2, :],
                                     pt2[:, 0, 256:512], start=False, stop=False,
                                     skip_group_check=True)
                    nc.tensor.matmul(yps[0:65, 256:512], vos[:, h, 4 * g + 3, :],
                                     pt2[:, 1, 256:512], start=False, stop=True,
                                     skip_group_check=True)
            return go

        return [grp(q) for q in range(ngr)]

    def epilogue(u):
        h, g = u

        def go():
            yps = yps_map.pop(u)
            ysb = ysbp.tile([65, 512], F32, name="ysb", tag="ysb")
            nc.vector.tensor_copy(out=ysb[:], in_=yps[0:65, :])
            # transpose back into the (drained) yps bank: tp = yps[:, 0:260]
            for k in range(4):
                nc.tensor.transpose(yps[:, k * 65:(k + 1) * 65],
                                    ysb[:, k * 128:(k + 1) * 128],
                                    identf[0:65, 0:65])
            rc4 = rcp.tile([128, 4], F32, name="rc4", tag="rc4")
            nc.vector.reciprocal(out=rc4[:], in_=yps[:, 64:260:65])
            nc.vector.tensor_scalar(out=rc4[:], in0=rc4[:], scalar1=kbc,
                                    scalar2=None, op0=OP.mult)
            yo = yop.tile([128, 4, 64], F32, name="yo", tag="yo")
            for k in range(4):
                nc.vector.scalar_tensor_tensor(
                    out=yo[:, k, :], in0=yps[:, k * 65:k * 65 + 64],
                    scalar=rc4[:, k:k + 1],
                    in1=static[:, 4 * g + k, h * 64:(h + 1) * 64],
                    op0=OP.mult, op1=OP.add)
            nc.sync.dma_start(
                out=yout[g * 512:(g + 1) * 512, h * 64:(h + 1) * 64]
                .rearrange("(k p) d -> p k d", p=128),
                in_=yo[:])
        return go

    # ---- pipelined emission ----
    slot = 0

    def maybe_proj(h):
        nonlocal proj_done, proj_pend, slot
        slot += 1
        want = min(h // 2 + 1, 3)
        if not proj_pend and proj_done <= want:
            proj_pend = list(proj_sched[proj_done])
            proj_done += 1
        if proj_pend and slot % 6 == 0:
            proj_pend.pop(0)()

    def drain_proj(p4need):
        nonlocal proj_done, proj_pend
        while proj_done <= p4need or (proj_pend and proj_done - 1 <= p4need):
            if not proj_pend:
                proj_pend = list(proj_sched[proj_done])
                proj_done += 1
            while proj_pend:
                proj_pend.pop(0)()

    prev_av = []
    for u in units:
        h, g = u
        drain_proj(h // 2)
        qk = qk_groups(u)
        n = max(len(qk), len(prev_av))
        for i in range(n):
            if i < len(qk):
                qk[i]()
            if i < len(prev_av):
                prev_av[i]()
            maybe_proj(h)
        prev_av = av_groups(u) + [epilogue(u)]
    for goav in prev_av:
        goav()

    for p in reversed(pools):
        p.release()


def build_nc():
    if "nc" in _NC_CACHE:
        return _NC_CACHE["nc"]
    nc = bacc.Bacc("TRN2", target_bir_lowering=False)
    xT = nc.declare_dram_parameter("xT", [128, 8 * 2048], BF16, isOutput=False)
    WT = nc.declare_dram_parameter("WT", [128, 8 * 1024], BF16, isOutput=False)
    vones = nc.declare_dram_parameter("vones", [128, NHC * NT * 65], BF16,
                                      isOutput=False)
    consb = nc.declare_dram_parameter("consb", [128, CB_W], BF16, isOutput=False)
    consf = nc.declare_dram_parameter("consf", [128, CF_W], F32, isOutput=False)
    yout = nc.declare_dram_parameter("yout", [T, 512], F32, isOutput=True)
    with tile.TileContext(nc) as tc:
        emit(nc, tc, xT, WT, vones, consb, consf, yout)
    nc.compile()
    _NC_CACHE["nc"] = nc
    return nc


def make_consts(alpha, beta, gamma):
    D = math.e + T - 1
    k1 = alpha * (math.e - 1.0) / D
    k2 = alpha / D
    jj = np.arange(128)
    trilm = (jj[:, None] <= jj[None, :]).astype(np.float32)
    cbf = np.zeros((128, CB_W), dtype=np.float32)
    cbf[:, CB_TRIL:CB_TRIL + 128] = trilm
    cbf[:, CB_ZTRIL + 128:CB_ZTRIL + 256] = trilm   # ztril = [0 | tril]
    cbf[:, CB_IDENT:CB_IDENT + 128] = np.eye(128, dtype=np.float32)
    cbf[:, CB_ONEC] = 1.0
    for I in range(17):
        JJ = np.arange(16)
        blk = (JJ[:, None] < I).astype(np.float32) * np.ones((16, 128), np.float32)
        cbf[0:16, CB_SEL + I * 128: CB_SEL + (I + 1) * 128] = blk
    consb = cbf.astype(BFNP)
    consf = np.zeros((128, CF_W), dtype=np.float32)
    for I in range(16):
        consf[:, I] = -gamma / (128.0 * I + jj + 1.0)
    consf[:, 16] = k1
    consf[:, 17] = k2
    consf[:, 18] = beta
    consf[:, CF_IDENT:CF_IDENT + 128] = np.eye(128, dtype=np.float32)
    return consb, consf


def kernel(x, w_attn, alpha, beta, gamma, _trace=False):
    x = np.asarray(x, dtype=np.float32)
    w_attn = np.asarray(w_attn, dtype=np.float32)
    alpha = float(np.asarray(alpha))
    beta = float(np.asarray(beta))
    gamma = float(np.asarray(gamma))

    nc = build_nc()
    consb, consf = make_consts(alpha, beta, gamma)
    in_maps = []
    for c in range(N_CORES):
        b, h0 = c // 2, (c % 2) * 8
        c0 = h0 * 64
        xb = x[b]
        # xT[p, ci, i] = xb[i, 128*ci + p]
        xT = np.ascontiguousarray(
            xb.T.reshape(8, 128, T).transpose(1, 0, 2)).astype(BFNP)
        wqk = np.concatenate([w_attn[c0:c0 + 512],
                              w_attn[C + c0:C + c0 + 512]], axis=0)  # [1024, C]
        # WT[p, ci, d] = wqk[d, 128*ci + p]
        WT = np.ascontiguousarray(
            wqk.T.reshape(8, 128, 1024).transpose(1, 0, 2)).astype(BFNP)
        # vones[p, hh, J, 0:64] = xb[J*128+p, c0+hh*64+d]; [..,64] = 1
        vsl = xb[:, c0:c0 + 512].reshape(NT, 128, NHC, 64).transpose(1, 2, 0, 3)
        vo = np.ones((128, NHC, NT, 65), dtype=np.float32)
        vo[:, :, :, 0:64] = vsl
        in_maps.append({
            "xT": xT.reshape(128, 8 * 2048),
            "WT": WT.reshape(128, 8 * 1024),
            "vones": vo.astype(BFNP).reshape(128, NHC * NT * 65),
            "consb": consb, "consf": consf,
        })
    res = run_bass_kernel_spmd(nc, in_maps, list(range(N_CORES)), trace=_trace)
    y = np.empty((B, T, C), dtype=np.float32)
    for c in range(N_CORES):
        b, h0 = c // 2, (c % 2) * 8
        y[b, :, h0 * 64: h0 * 64 + 512] = res.results[c]["yout"]
    if _trace:
        kernel.last_exec_time_ns = res.exec_time_ns
    return y


# revision 11
# speedup vs baseline: 2.1138x; 1.1703x over previous
"""Causal shaped attention kernel for Trainium2 (8 NeuronCores).

y = beta * softmax(causal(q k^T / 8)) @ v + alpha * Id @ v - gamma * MC @ v
  with q,k = x @ w_attn.T split, v = x, Id = softmax(eye(T)), MC = causal row-mean.

Sharding: (batch, head-group) across 8 cores: core c -> b = c//2, heads
h0 = (c%2)*8 .. h0+8.  Each core computes y[b, :, h0*64 : h0*64+512].

Id@v and MC@v have closed forms (no T x T materialization):
  Id@v[i] = ((e-1) v[i] + colsum(v)) / (e+T-1)
  MC@v[i] = cumsum(v)[i] / (i+1)

v2 design (vs v1 baseline at ~583us):
 - x^T, W^T and the vones ([v|1]) AV-lhsT are packed on HOST in bf16:
   no on-device transposes, half the input DMA bytes, half the LDWEIGHTS.
 - B2 (Id/MC "static" term) has no serial prefix recurrence: per-tile
   column sums go into a [16,512] PSUM tile (one matmul per row), then a
   per-tile exclusive prefix is formed by 16 independent selector matmuls
   (selcol[J, I*128+p] = J<I), + one tril matmul for the local cumsum.
 - Phase C is software-pipelined at tile-pair granularity: the QK+exp
   stream of unit u+1 interleaves with the AV stream of unit u, and
   projection chains for head pair p+1 are injected into the PE stream
   while heads of pair p run attention, keeping PE dense (p-state!).
 - Everything 2-byte on the PE: qkT, pt (exp output), vones in bf16;
   PSUM stays fp32.
"""

import sys

if "/opt/trn_rl_repo" not in sys.path:
    sys.path.insert(0, "/opt/trn_rl_repo")

import math

import numpy as np
import ml_dtypes

import concourse.bass as bass
import concourse.mybir as mybir
import concourse.tile as tile
from concourse import bacc
from concourse.bass_utils import run_bass_kernel_spmd

F32 = mybir.dt.float32
BF16 = mybir.dt.bfloat16
AF = mybir.ActivationFunctionType
OP = mybir.AluOpType
BFNP = ml_dtypes.bfloat16

N_CORES = 8
B, T, C = 4, 2048, 1024
H, HD = 16, 64
NHC = 8          # heads per core
NT = T // 128    # 16 j/i tiles
NS = 4           # i-strips of 512

# bf16 consts layout: tril 128 | ztril 256 | ident 128 | ones_col 1 | selcol 17*128
CB_TRIL = 0
CB_ZTRIL = 128
CB_IDENT = 384
CB_ONEC = 512
CB_SEL = 513
CB_W = 513 + 17 * 128          # 2689
# f32 consts: negipg 16 | k1 | k2 | kb | pad | ident128
CF_IDENT = 20
CF_W = 20 + 128

_NC_CACHE = {}


def emit(nc, tc, xT, WT, vones, consb, consf, yout):
    pools = []

    def pool(name, **kw):
        p = tc.alloc_tile_pool(name=name, **kw)
        pools.append(p)
        return p

    # ---- persistent SBUF ----
    cpool = pool("cpool", bufs=1)
    cb = cpool.tile([128, CB_W], BF16, name="cb")
    cf = cpool.tile([128, CF_W], F32, name="cf")
    xTs = cpool.tile([128, 8, 2048], BF16, name="xTs")
    WTs = cpool.tile([128, 8, 1024], BF16, name="WTs")
    vos = cpool.tile([128, NHC, NT, 65], BF16, name="vos")
    qkT = cpool.tile([128, 4, 2, 2048], BF16, name="qkT")
    static = cpool.tile([128, NT, 512], F32, name="static")
    colb = cpool.tile([128, 512], F32, name="colb")
    run = cpool.tile([1, 512], BF16, name="run")

    nc.sync.dma_start(out=cb[:], in_=consb[:])
    nc.sync.dma_start(out=cf[:], in_=consf[:])
    nc.sync.dma_start(out=vos[:], in_=vones[:].rearrange(
        "p (h J d) -> p h J d", h=NHC, J=NT))
    nc.sync.dma_start(out=WTs[:], in_=WT[:].rearrange("p (c d) -> p c d", c=8))
    nc.sync.dma_start(out=xTs[:], in_=xT[:].rearrange("p (c i) -> p c i", c=8))

    tril = cb[:, CB_TRIL:CB_TRIL + 128]
    ztril = cb[:, CB_ZTRIL:CB_ZTRIL + 256]
    identf = cf[:, CF_IDENT:CF_IDENT + 128]
    ones_col = cb[:, CB_ONEC:CB_ONEC + 1]
    k1c = cf[:, 16:17]
    k2c = cf[:, 17:18]
    kbc = cf[:, 18:19]

    def selcol(i):
        return cb[0:16, CB_SEL + i * 128: CB_SEL + (i + 1) * 128]

    # ---- PSUM pools: proj pj (2 banks) + B2 (5, released before phase C) ----
    pspj = pool("pspj", bufs=2, space="PSUM")
    psB2 = pool("psB2", bufs=1, space="PSUM")

    # ================= B2: static = k1*v + k2*colsum - g/(i+1)*cumsum ======
    ones_row = tril[0:1, 0:128]   # tril row 0 == all ones (K=1 lhsT)
    nc.vector.memset(run[:], 0.0)
    # per-tile column sums (one matmul each), exclusive tile-prefix kept as a
    # rolling [1,512] bf16 row (DVE in-place adds; /(i+1) makes rounding moot)
    for I in range(NT):
        cp = psB2.tile([1, 512], F32, name="cp", tag="cp", bufs=2)
        nc.tensor.matmul(cp[0:1, :], ones_col, vos[:, :, I, 0:64],
                         start=True, stop=True)
        cu = psB2.tile([128, 512], F32, name="cu", tag="cu", bufs=2)
        nc.tensor.matmul(cu[:], ones_row, run[0:1, :], start=True, stop=False)
        nc.tensor.matmul(cu[:], tril, vos[:, :, I, 0:64], start=False, stop=True)
        nc.vector.tensor_add(run[0:1, :], run[0:1, :], cp[0:1, :])
        # static_I = negipg_I * cu  (cu bank recycled 2 tiles later)
        nc.vector.tensor_scalar(out=static[:, I, :], in0=cu[:],
                                scalar1=cf[:, I:I + 1], scalar2=None,
                                op0=OP.mult)
    colb_ps = psB2.tile([128, 512], F32, name="colb_ps", tag="colb")
    nc.tensor.matmul(colb_ps[:], ones_row, run[0:1, :], start=True, stop=True)
    nc.vector.tensor_scalar(out=colb[:], in0=colb_ps[:], scalar1=k2c,
                            scalar2=None, op0=OP.mult)
    for I in range(NT):
        # static_I += k1*v_I
        nc.vector.scalar_tensor_tensor(
            out=static[:, I, :].rearrange("p (h d) -> p h d", h=NHC),
            in0=vos[:, :, I, 0:64], scalar=k1c,
            in1=static[:, I, :].rearrange("p (h d) -> p h d", h=NHC),
            op0=OP.mult, op1=OP.add)
    for I in range(NT):
        # static_I += colb
        nc.vector.tensor_add(static[:, I, :], static[:, I, :], colb[:])
    psB2.release()
    pools.remove(psB2)

    # ================= projection groups =================
    # group (p4, qk, s): 8-matmul K-chain into pj, DVE-evacuate to qkT (bf16)
    def proj_group(p4, qk, s):
        def go():
            pj = pspj.tile([128, 512], F32, name="pj", tag="pj")
            for ci in range(8):
                nc.tensor.matmul(
                    pj[:], WTs[:, ci, qk * 512 + p4 * 128: qk * 512 + (p4 + 1) * 128],
                    xTs[:, ci, s * 512:(s + 1) * 512],
                    start=(ci == 0), stop=(ci == 7))
            dst = qkT[:, p4, qk, s * 512:(s + 1) * 512]
            if p4 == 0:
                nc.scalar.copy(out=dst, in_=pj[:])
            else:
                nc.vector.tensor_copy(out=dst, in_=pj[:])
        return go

    proj_sched = [[proj_group(p4, qk, s) for qk in range(2) for s in range(NS)]
                  for p4 in range(4)]

    # emit pair 0 projection up-front
    for go in proj_sched[0]:
        go()
    proj_done = 1  # pairs fully emitted
    proj_pend = []  # pending closures for pair proj_done (being drip-fed)

    # ================= phase C =================
    psst = pool("psst", bufs=2, space="PSUM")   # st2 [128,2,512] -> 4 banks
    psyp = pool("psyp", bufs=2, space="PSUM")   # yps [128,512]   -> 2 banks
    ptp = pool("ptp", bufs=10)                  # pt2 ring (bf16)
    ysbp = pool("ysbp", bufs=2)
    rcp = pool("rcp", bufs=2)
    yop = pool("yop", bufs=2)

    units = [(h, g) for h in range(NHC) for g in range(NS)]

    def qk_groups(u):
        """List of closures; each emits 2 QK matmuls + 1 exp (+ masks) and
        returns the pt2 tile for the AV stage (stored into pt_map)."""
        h, g = u
        p4, base = h // 2, (h % 2) * 64
        qT = qkT[base:base + 64, p4, 0, :]
        kT = qkT[base:base + 64, p4, 1, :]
        i0 = g * 512
        out = []

        def full_pair(fp):
            def go():
                st2 = psst.tile([128, 2, 512], F32, name="st2", tag="st2")
                for u2 in range(2):
                    J = 2 * fp + u2
                    nc.tensor.matmul(st2[:, u2, :], kT[:, J * 128:(J + 1) * 128],
                                     qT[:, i0:i0 + 512], start=True, stop=True,
                                     skip_group_check=True)
                pt2 = ptp.tile([128, 2, 512], BF16, name="pt2", tag="pt2")
                nc.scalar.activation(out=pt2[:], in_=st2[:], func=AF.Exp,
                                     scale=0.125)
                pt_map[(u, fp)] = pt2
            return go

        def diagA():
            def go():
                st2 = psst.tile([128, 2, 512], F32, name="st2", tag="st2")
                J = 4 * g
                nc.tensor.matmul(st2[:, 0, :], kT[:, J * 128:(J + 1) * 128],
                                 qT[:, i0:i0 + 512], start=True, stop=True,
                                 skip_group_check=True)
                nc.tensor.matmul(st2[:, 1, 128:512],
                                 kT[:, (J + 1) * 128:(J + 2) * 128],
                                 qT[:, i0 + 128:i0 + 512], start=True, stop=True,
                                 skip_group_check=True)
                pt2 = ptp.tile([128, 2, 512], BF16, name="pt2", tag="pt2")
                # whole [2,512] exp'd in one inst; [1, 0:128] is stale junk
                # (finite) that AV never reads.
                nc.scalar.activation(out=pt2[:], in_=st2[:], func=AF.Exp,
                                     scale=0.125)
                nc.vector.tensor_mul(pt2[:, 0, 0:128], pt2[:, 0, 0:128], tril)
                nc.vector.tensor_mul(pt2[:, 1, 128:256], pt2[:, 1, 128:256], tril)
                pt_map[(u, 2 * g)] = pt2
            return go

        def diagB():
            def go():
                st2 = psst.tile([128, 2, 512], F32, name="st2", tag="st2")
                for u2 in range(2):
                    J = 4 * g + 2 + u2
                    nc.tensor.matmul(st2[:, u2, 256:512],
                                     kT[:, J * 128:(J + 1) * 128],
                                     qT[:, i0 + 256:i0 + 512], start=True,
                                     stop=True, skip_group_check=True)
                pt2 = ptp.tile([128, 2, 512], BF16, name="pt2", tag="pt2")
                nc.scalar.activation(out=pt2[:, :, 256:512],
                                     in_=st2[:, :, 256:512], func=AF.Exp,
                                     scale=0.125)
                nc.vector.tensor_mul(pt2[:, 0, 256:384], pt2[:, 0, 256:384], tril)
                nc.vector.tensor_mul(pt2[:, 1, 256:512], pt2[:, 1, 256:512], ztril)
                pt_map[(u, 2 * g + 1)] = pt2
            return go

        for fp in range(2 * g):
            out.append(full_pair(fp))
        out.append(diagA())
        out.append(diagB())
        return out

    pt_map = {}
    yps_map = {}

    def av_groups(u):
        h, g = u
        ngr = 2 * g + 2

        def grp(q):
            def go():
                if q == 0:
                    yps_map[u] = psyp.tile([128, 512], F32, name="yps", tag="yps")
                yps = yps_map[u]
                pt2 = pt_map.pop((u, q))
                if q < 2 * g:          # full pair
                    for u2 in range(2):
                        J = 2 * q + u2
                        nc.tensor.matmul(yps[0:65, :], vos[:, h, J, :],
                                         pt2[:, u2, :],
                                         start=(q == 0 and u2 == 0),
                                         stop=(q == ngr - 1 and u2 == 1),
                                         skip_group_check=True)
                elif q == 2 * g:       # diagA
                    nc.tensor.matmul(yps[0:65, :], vos[:, h, 4 * g, :],
                                     pt2[:, 0, :], start=(q == 0), stop=False,
                                     skip_group_check=True)
                    nc.tensor.matmul(yps[0:65, 128:512], vos[:, h, 4 * g + 1, :],
                                     pt2[:, 1, 128:512], start=False, stop=False,
                                     skip_group_check=True)
                else:                  # diagB
                    nc.tensor.matmul(yps[0:65, 256:512], vos[:, h, 4 * g + 2, :],
                                     pt2[:, 0, 256:512], start=False, stop=False,
                                     skip_group_check=True)
                    nc.tensor.matmul(yps[0:65, 256:512], vos[:, h, 4 * g + 3, :],
                                     pt2[:, 1, 256:512], start=False, stop=True,
                                     skip_group_check=True)
            return go

        return [grp(q) for q in range(ngr)]

    def epilogue(u):
        h, g = u

        def go():
            yps = yps_map.pop(u)
            ysb = ysbp.tile([65, 512], F32, name="ysb", tag="ysb")
            nc.vector.tensor_copy(out=ysb[:], in_=yps[0:65, :])
            # transpose back into the (drained) yps bank: tp = yps[:, 0:260]
            for k in range(4):
                nc.tensor.transpose(yps[:, k * 65:(k + 1) * 65],
                                    ysb[:, k * 128:(k + 1) * 128],
                                    identf[0:65, 0:65])
            rc4 = rcp.tile([128, 4], F32, name="rc4", tag="rc4")
            nc.vector.reciprocal(out=rc4[:], in_=yps[:, 64:260:65])
            nc.vector.tensor_scalar(out=rc4[:], in0=rc4[:], scalar1=kbc,
                                    scalar2=None, op0=OP.mult)
            yo = yop.tile([128, 4, 64], F32, name="yo", tag="yo")
            for k in range(4):
                nc.vector.scalar_tensor_tensor(
                    out=yo[:, k, :], in0=yps[:, k * 65:k * 65 + 64],
                    scalar=rc4[:, k:k + 1],
                    in1=static[:, 4 * g + k, h * 64:(h + 1) * 64],
                    op0=OP.mult, op1=OP.add)
            nc.sync.dma_start(
                out=yout[g * 512:(g + 1) * 512, h * 64:(h + 1) * 64]
                .rearrange("(k p) d -> p k d", p=128),
                in_=yo[:])
        return go

    # ---- pipelined emission ----
    slot = 0

    def maybe_proj(h):
        nonlocal proj_done, proj_pend, slot
        slot += 1
        want = min(h // 2 + 1, 3)
        if not proj_pend and proj_done <= want:
            proj_pend = list(proj_sched[proj_done])
            proj_done += 1
        if proj_pend and slot % 6 == 0:
            proj_pend.pop(0)()

    def drain_proj(p4need):
        nonlocal proj_done, proj_pend
        while proj_done <= p4need or (proj_pend and proj_done - 1 <= p4need):
            if not proj_pend:
                proj_pend = list(proj_sched[proj_done])
                proj_done += 1
            while proj_pend:
                proj_pend.pop(0)()

    prev_av = []
    for u in units:
        h, g = u
        drain_proj(h // 2)
        qk = qk_groups(u)
        n = max(len(qk), len(prev_av))
        for i in range(n):
            if i < len(qk):
                qk[i]()
            if i < len(prev_av):
                prev_av[i]()
            maybe_proj(h)
        prev_av = av_groups(u) + [epilogue(u)]
    for goav in prev_av:
        goav()

    for p in reversed(pools):
        p.release()


def build_nc():
    if "nc" in _NC_CACHE:
        return _NC_CACHE["nc"]
    nc = bacc.Bacc("TRN2", target_bir_lowering=False)
    xT = nc.declare_dram_parameter("xT", [128, 8 * 2048], BF16, isOutput=False)
    WT = nc.declare_dram_parameter("WT", [128, 8 * 1024], BF16, isOutput=False)
    vones = nc.declare_dram_parameter("vones", [128, NHC * NT * 65], BF16,
                                      isOutput=False)
    consb = nc.declare_dram_parameter("consb", [128, CB_W], BF16, isOutput=False)
    consf = nc.declare_dram_parameter("consf", [128, CF_W], F32, isOutput=False)
    yout = nc.declare_dram_parameter("yout", [T, 512], F32, isOutput=True)
    with tile.TileContext(nc) as tc:
        emit(nc, tc, xT, WT, vones, consb, consf, yout)
    nc.compile()
    _NC_CACHE["nc"] = nc
    return nc


def make_consts(alpha, beta, gamma):
    D = math.e + T - 1
    k1 = alpha * (math.e - 1.0) / D
    k2 = alpha / D
    jj = np.arange(128)
    trilm = (jj[:, None] <= jj[None, :]).astype(np.float32)
    cbf = np.zeros((128, CB_W), dtype=np.float32)
    cbf[:, CB_TRIL:CB_TRIL + 128] = trilm
    cbf[:, CB_ZTRIL + 128:CB_ZTRIL + 256] = trilm   # ztril = [0 | tril]
    cbf[:, CB_IDENT:CB_IDENT + 128] = np.eye(128, dtype=np.float32)
    cbf[:, CB_ONEC] = 1.0
    for I in range(17):
        JJ = np.arange(16)
        blk = (JJ[:, None] < I).astype(np.float32) * np.ones((16, 128), np.float32)
        cbf[0:16, CB_SEL + I * 128: CB_SEL + (I + 1) * 128] = blk
    consb = cbf.astype(BFNP)
    consf = np.zeros((128, CF_W), dtype=np.float32)
    for I in range(16):
        consf[:, I] = -gamma / (128.0 * I + jj + 1.0)
    consf[:, 16] = k1
    consf[:, 17] = k2
    consf[:, 18] = beta
    consf[:, CF_IDENT:CF_IDENT + 128] = np.eye(128, dtype=np.float32)
    return consb, consf


def kernel(x, w_attn, alpha, beta, gamma, _trace=False):
    x = np.asarray(x, dtype=np.float32)
    w_attn = np.asarray(w_attn, dtype=np.float32)
    alpha = float(np.asarray(alpha))
    beta = float(np.asarray(beta))
    gamma = float(np.asarray(gamma))

    nc = build_nc()
    consb, consf = make_consts(alpha, beta, gamma)
    in_maps = []
    for c in range(N_CORES):
        b, h0 = c // 2, (c % 2) * 8
        c0 = h0 * 64
        xb = x[b]
        # xT[p, ci, i] = xb[i, 128*ci + p]
        xT = np.ascontiguousarray(
            xb.T.reshape(8, 128, T).transpose(1, 0, 2)).astype(BFNP)
        wqk = np.concatenate([w_attn[c0:c0 + 512],
                              w_attn[C + c0:C + c0 + 512]], axis=0)  # [1024, C]
        # WT[p, ci, d] = wqk[d, 128*ci + p]
        WT = np.ascontiguousarray(
            wqk.T.reshape(8, 128, 1024).transpose(1, 0, 2)).astype(BFNP)
        # vones[p, hh, J, 0:64] = xb[J*128+p, c0+hh*64+d]; [..,64] = 1
        vsl = xb[:, c0:c0 + 512].reshape(NT, 128, NHC, 64).transpose(1, 2, 0, 3)
        vo = np.ones((128, NHC, NT, 65), dtype=np.float32)
        vo[:, :, :, 0:64] = vsl
        in_maps.append({
            "xT": xT.reshape(128, 8 * 2048),
            "WT": WT.reshape(128, 8 * 1024),
            "vones": vo.astype(BFNP).reshape(128, NHC * NT * 65),
            "consb": consb, "consf": consf,
        })
    res = run_bass_kernel_spmd(nc, in_maps, list(range(N_CORES)), trace=_trace)
    y = np.empty((B, T, C), dtype=np.float32)
    for c in range(N_CORES):
        b, h0 = c // 2, (c % 2) * 8
        y[b, :, h0 * 64: h0 * 64 + 512] = res.results[c]["yout"]
    if _trace:
        kernel.last_exec_time_ns = res.exec_time_ns
    return y
